# revision 89
# baseline (speedup 1.0000x reference)
"""MultiHeadAttention Trainium2 kernel (8 NeuronCores, SPMD).

Reference computation (B=4, T=1024, D=768, H=12, Dh=64):
    q = x @ Wq.T ; k = x @ Wk.T ; v = x @ Wv.T       (per-head reshape)
    attn = softmax((q @ k.T) / 8)
    out = (attn @ v) @ Wo.T + bo

Sharding: 8 cores = 4 batches x 2 head-halves (6 heads each). Each core
emits a [1024, 768] output-projection partial for its 6 heads (rows
768:1024 via a separate bf16 tensor to shrink the tail DMAs); the host
sums the two partials per batch and adds the bias.

All device data is bf16 (host-converted into SBUF-layout DRAM images;
small per-chunk DMAs are coalesced because HWDGE issue (625ns) exceeds
their transfer time), fp32 PSUM accumulation. bf16 matmuls run at
1 cycle/moving-row for any tile width, which enables the narrow-N
context matmul below; DMA bytes halve.

Per-core dataflow:
    qT,kT = (W x) in [dh(384), t] layout, one m-chunk of 128 = 2 heads;
        the first chunk's matmuls chase the x token-halves as they land
    v     = (x Wv) in [t, 6*(64+1)] tiles; col 64 of each head block is
        ones so the ctx matmul also emits the softmax denominators
    S.T psum [kt(4x128), q(256)] = kT_h.T @ qT_h      (K=64, N=256)
    expS  = exp(S.T) bf16, ScalarE reading PSUM in [128, 1024] tiles.
        The exp chain is the critical path (48 tiles x ~1.04us): its
        feeder matmuls run in a high-priority scheduler band with 3
        PSUM tiles in flight, head-major over query-halves qq0/qq1
        (startup then needs only the m0 weights) and quarter-major for
        qq2/qq3 so finished output columns retire early and only
        quarter qq3's ctx/out work sits on the tail.
    ctx[q, 65] psum += expS_j.T @ [v_j | 1]           (K=kt, N=65)
        col 64 = denominator; DVE reciprocal + per-partition
        tensor_scalar_mul normalizes (denominator is per-q = per-row)
    ctxT: DMA-XBAR sbuf->sbuf bf16 transposes, emitted BEFORE the prior
        quarter's output DMAs so they never queue behind them on
        SP/HWDGE; the tail pair goes through the PE (transpose-matmul,
        psum borrowed from the then-idle scores pool, ScalarE copies)
    out[q, :] = sum_m ctxT_m.T @ Wo_m  (3-chunk psum accumulation)
"""
import numpy as np
import ml_dtypes

import concourse.mybir as mybir
from concourse import bacc
from concourse.tile import TileContext
from concourse.bass_utils import run_bass_kernel_spmd

FP = mybir.dt.float32
BF = mybir.dt.bfloat16
AF = mybir.ActivationFunctionType
BF_NP = ml_dtypes.bfloat16

B, T, D = 4, 1024, 768
H, DH = 12, 64
NCORES = 8
HPC = 6           # heads per core
DPC = HPC * DH    # 384 head-dims per core
KC = D // 128     # 6 contraction chunks of d_in
MC = DPC // 128   # 3 chunks of per-core head dims (2 heads each)
QC = T // 128     # 8 query chunks
TT = T // 128     # 8 key chunks


def emit_mha(tc, xt, wk, wq, wv, wo, ident, o1d, o2d, ctx):
    nc = tc.nc

    # Priority bands for the Tile scheduler (dynamic, per-engine, lower =
    # preferred among READY instructions). The exp chain on ScalarE is the
    # critical path: its feeders (chase + scores matmuls) outrank the
    # projections, which outrank ctx/normalize, which outrank transposes
    # and the output projection. Readiness gating keeps the tail correct.
    from contextlib import contextmanager

    B_CHAIN, B_PROJ, B_CTX, B_OUT = -5_000_000, None, 500_000, None
    B_XBAR = -1_000_000
    band_next = {B_CTX: B_CTX, B_CHAIN: B_CHAIN, B_XBAR: B_XBAR}

    @contextmanager
    def prio(band):
        if band is None:
            yield
            return
        saved = tc.cur_priority
        tc.cur_priority = band_next[band]
        try:
            yield
        finally:
            band_next[band] = tc.cur_priority
            tc.cur_priority = saved

    singles = ctx.enter_context(tc.tile_pool(name="singles", bufs=1))
    # scores psum: [128,1024] fp32 = 2 banks each; 2 bufs = 4 banks
    sps = ctx.enter_context(tc.tile_pool(name="sps", bufs=3, space="PSUM"))
    # shared work psum (qk/v/ctx/transpose/out): 4 bufs x 1 bank = 4 banks
    wps = ctx.enter_context(tc.tile_pool(name="wps", bufs=2, space="PSUM"))
    expp = ctx.enter_context(tc.tile_pool(name="expp", bufs=34))
    osbp = ctx.enter_context(tc.tile_pool(name="osbp", bufs=8))

    # ---------------- SBUF singles ----------------
    xT_sb = singles.tile([128, KC, T], BF, name="xT_sb", tag="xT_sb")
    wk_sb = singles.tile([128, MC, 768], BF, name="wk_sb", tag="wk_sb")
    wq_sb = singles.tile([128, MC, 768], BF, name="wq_sb", tag="wq_sb")
    wv_sb = singles.tile([128, KC, DPC], BF, name="wv_sb", tag="wv_sb")
    wo_sb = singles.tile([128, MC, 768], BF, name="wo_sb", tag="wo_sb")
    id_sb = singles.tile([128, 128], BF, name="id_sb", tag="id_sb")
    kT_sb = singles.tile([128, MC, T], BF, name="kT_sb", tag="kT_sb")
    qT_sb = singles.tile([128, MC, T], BF, name="qT_sb", tag="qT_sb")
    ctxn_sb = singles.tile([128, QC, DPC], BF, name="ctxn_sb", tag="ctxn_sb")
    ctxT_sb = singles.tile([128, MC, T], BF, name="ctxT_sb", tag="ctxT_sb")
    rcp_sb = singles.tile([128, HPC * QC], FP, name="rcp_sb", tag="rcp_sb")
    v_sb = []
    for j in range(TT):
        vt = singles.tile([128, HPC, DH + 1], BF, name=f"v_sb{j}", tag=f"v_sb{j}")
        v_sb.append(vt)

    # ones columns for the fused softmax denominators (Pool engine, SBUF)
    for j in range(TT):
        nc.gpsimd.memset(v_sb[j][:, :, DH : DH + 1], 1.0)

    # ---------------- input DMAs (SP/HWDGE queue) ----------------
    xtr = xt.rearrange("p (c t) -> p c t", c=KC)
    nc.sync.dma_start(out=wk_sb[:, 0, :], in_=wk[:, 0:768])
    nc.sync.dma_start(out=xT_sb[:, 0, 0:512], in_=xtr[:, 0, 0:512])
    nc.sync.dma_start(out=wq_sb[:, 0, :], in_=wq[:, 0:768])
    # coalesced: per-chunk DMAs would be HWDGE-issue-limited (625ns each
    # vs 364ns transfers), so the later chunks land sooner as one transfer
    nc.sync.dma_start(out=xT_sb[:, 1:KC, 0:512], in_=xtr[:, 1:KC, 0:512])
    nc.sync.dma_start(out=xT_sb[:, :, 512:1024], in_=xtr[:, :, 512:1024])
    nc.sync.dma_start(out=wk_sb[:, 1:3, :], in_=wk[:, 768:2304])
    nc.sync.dma_start(out=wq_sb[:, 1:3, :], in_=wq[:, 768:2304])
    nc.sync.dma_start(out=wv_sb, in_=wv.rearrange("p (c n) -> p c n", c=KC))
    nc.sync.dma_start(out=wo_sb, in_=wo.rearrange("p (m d) -> p m d", m=MC))
    nc.sync.dma_start(out=id_sb, in_=ident)

    # ---------------- atoms ----------------
    expS = {}

    def qk_half(m, n, w_sb, dst, act_copy, band=B_PROJ):
      with prio(band):
        ps = wps.tile([128, 512], FP, name="ps_qk", tag="w")
        for c in range(KC):
            nc.tensor.matmul(
                ps,
                lhsT=w_sb[:, m, c * 128 : (c + 1) * 128],
                rhs=xT_sb[:, c, n * 512 : (n + 1) * 512],
                start=(c == 0),
                stop=(c == KC - 1),
            )
        if act_copy:
            nc.scalar.copy(dst[:, m, n * 512 : (n + 1) * 512], ps)
        else:
            nc.vector.tensor_copy(dst[:, m, n * 512 : (n + 1) * 512], ps)

    def score(h, qq, jq):
      with prio(B_CHAIN):
        # S.T for key chunks j = 4jq..4jq+3, query quarter qq, one head
        m, po = h // 2, 64 * (h % 2)
        ps = sps.tile([128, 1024], FP, name="ps_s", tag="s")
        for r in range(4):
            j = 4 * jq + r
            nc.tensor.matmul(
                ps[:, r * 256 : (r + 1) * 256],
                lhsT=kT_sb[po : po + 64, m, j * 128 : (j + 1) * 128],
                rhs=qT_sb[po : po + 64, m, qq * 256 : (qq + 1) * 256],
                start=True,
                stop=True,
            )
        ex = expp.tile([128, 1024], BF, name="ex", tag="ex")
        nc.scalar.activation(ex, ps, AF.Exp)
        expS[(h, qq, jq)] = ex

    def v_mt(mt):
      with prio(B_PROJ):
        ps = wps.tile([128, DPC], FP, name="ps_v", tag="w")
        for c in range(KC):
            nc.tensor.matmul(
                ps,
                lhsT=xT_sb[:, c, mt * 128 : (mt + 1) * 128],
                rhs=wv_sb[:, c, :],
                start=(c == 0),
                stop=(c == KC - 1),
            )
        nc.vector.tensor_copy(v_sb[mt][:, :, 0:DH], ps)

    def ctx_pair(pair, qc):
      with prio(B_CTX):
        # ctx[q, dh|denom] for heads 2p,2p+1 in one psum tile [128, 130]
        pc = wps.tile([128, 130], FP, name="pc", tag="w")
        for hi in range(2):
            h = 2 * pair + hi
            col = hi * 65
            for j in range(TT):
                ex = expS[(h, qc // 2, j // 4)]
                off = (j % 4) * 256 + (qc % 2) * 128
                nc.tensor.matmul(
                    pc[:, col : col + 65],
                    lhsT=ex[:, off : off + 128],
                    rhs=v_sb[j][:, h, :],
                    start=(j == 0),
                    stop=(j == TT - 1),
                )
      with prio(B_CTX):
        for hi in range(2):
            h = 2 * pair + hi
            k = h * QC + qc
            nc.vector.reciprocal(
                rcp_sb[:, k : k + 1], pc[:, hi * 65 + 64 : hi * 65 + 65]
            )
            nc.vector.tensor_scalar_mul(
                ctxn_sb[:, qc, h * 64 : (h + 1) * 64],
                pc[:, hi * 65 : hi * 65 + 64],
                rcp_sb[:, k : k + 1],
            )

    def tpose_dma(pair, qc):
      with prio(B_XBAR):
        # DMA-XBAR sbuf->sbuf bf16 transpose, off the PE/DVE path
        nc.sync.dma_start_transpose(
            out=ctxT_sb[:, pair, qc * 128 : (qc + 1) * 128],
            in_=ctxn_sb[:, qc, pair * 128 : (pair + 1) * 128],
        )

    def tpose_pe(pair, qc):
      with prio(B_OUT):
        # tail-only: borrow the scores psum slots (free after the last exp)
        tp = sps.tile([128, 128], BF, name="tp", tag="s")
        nc.tensor.matmul(
            tp,
            lhsT=ctxn_sb[:, qc, pair * 128 : (pair + 1) * 128],
            rhs=id_sb,
            is_transpose=True,
        )
        nc.scalar.copy(ctxT_sb[:, pair, qc * 128 : (qc + 1) * 128], tp)

    def out_full(qc, tail=False):
      with prio(B_OUT):
        # tail rows go out as bf16 (host upcasts): halves the final DMAs
        osb = osbp.tile([128, D], BF if tail else FP, name="osb1", tag="osb1")
        for n2 in range(2):
            pool, tag = (sps, "s") if tail else (wps, "w")
            ps = pool.tile([128, 384], FP, name="ps_o", tag=tag)
            for m in range(MC):
                nc.tensor.matmul(
                    ps,
                    lhsT=ctxT_sb[:, m, qc * 128 : (qc + 1) * 128],
                    rhs=wo_sb[:, m, n2 * 384 : (n2 + 1) * 384],
                    start=(m == 0),
                    stop=(m == MC - 1),
                )
            if tail and n2 == 1:
                nc.scalar.copy(osb[:, n2 * 384 : (n2 + 1) * 384], ps)
            else:
                nc.vector.tensor_copy(osb[:, n2 * 384 : (n2 + 1) * 384], ps)
        if tail:
            nc.sync.dma_start(
                out=o2d[(qc - 6) * 128 : (qc - 5) * 128, :], in_=osb
            )
        else:
            nc.sync.dma_start(out=o1d[qc * 128 : (qc + 1) * 128, :], in_=osb)

    # ---------------- schedule ----------------
    # The ScalarE exp chain is the clock. Hybrid order: head-major over
    # query-halves qq0/qq1 first (startup then only needs the m0 weight
    # block), quarter-major for qq2/qq3 so complete output columns retire
    # early and only quarter qq3 sits on the tail.
    qk_half(0, 0, wk_sb, kT_sb, act_copy=True, band=B_CHAIN)
    qk_half(0, 0, wq_sb, qT_sb, act_copy=True, band=B_CHAIN)
    score(0, 0, 0)
    qk_half(0, 1, wk_sb, kT_sb, act_copy=False, band=B_CHAIN)
    score(0, 0, 1)
    score(1, 0, 0)
    qk_half(0, 1, wq_sb, qT_sb, act_copy=False, band=B_CHAIN)
    score(1, 0, 1)
    for h in (0, 1):
        score(h, 1, 0)
        score(h, 1, 1)
    for n in range(2):
        qk_half(1, n, wk_sb, kT_sb, act_copy=False)
        qk_half(1, n, wq_sb, qT_sb, act_copy=False)
    score(2, 0, 0)
    score(2, 0, 1)
    v_mt(0)
    v_mt(1)
    score(3, 0, 0)
    score(3, 0, 1)
    v_mt(2)
    v_mt(3)
    score(2, 1, 0)
    score(2, 1, 1)
    v_mt(4)
    v_mt(5)
    score(3, 1, 0)
    score(3, 1, 1)
    v_mt(6)
    v_mt(7)
    for n in range(2):
        qk_half(2, n, wk_sb, kT_sb, act_copy=False)
        qk_half(2, n, wq_sb, qT_sb, act_copy=False)
    for h in (4, 5):
        score(h, 0, 0)
        score(h, 0, 1)
    for qc in range(4):
        ctx_pair(0, qc)
        tpose_dma(0, qc)
    score(4, 1, 0)
    score(4, 1, 1)
    score(5, 1, 0)
    score(5, 1, 1)
    for qc in range(4):
        ctx_pair(1, qc)
        tpose_dma(1, qc)
    for qc in range(4):
        ctx_pair(2, qc)
        tpose_dma(2, qc)
    for qc in range(4):
        out_full(qc)
    # quarter qq2
    for pair in range(2):
        for h in (2 * pair, 2 * pair + 1):
            score(h, 2, 0)
            score(h, 2, 1)
        for qc in (4, 5):
            ctx_pair(pair, qc)
            tpose_dma(pair, qc)
    for h in (4, 5):
        score(h, 2, 0)
        score(h, 2, 1)
    for qc in (4, 5):
        ctx_pair(2, qc)
        tpose_dma(2, qc)
    # quarter qq3: its pair-0/1 XBAR transposes are emitted BEFORE the
    # qq2 output DMAs so they never queue behind them on SP/HWDGE
    for pair in range(2):
        for h in (2 * pair, 2 * pair + 1):
            score(h, 3, 0)
            score(h, 3, 1)
        for qc in (6, 7):
            ctx_pair(pair, qc)
            tpose_dma(pair, qc)
    out_full(4)
    out_full(5)
    for h in (4, 5):
        score(h, 3, 0)
        score(h, 3, 1)
    for qc in (6, 7):
        ctx_pair(2, qc)
        tpose_pe(2, qc)
    out_full(6, tail=True)
    out_full(7, tail=True)


_PROGRAM = None


def build_program():
    global _PROGRAM
    if _PROGRAM is not None:
        return _PROGRAM
    nc = bacc.Bacc("TRN2", target_bir_lowering=False, debug=False, num_devices=NCORES)
    xt = nc.dram_tensor("xt", (128, KC * T), BF, kind="ExternalInput").ap()
    wk = nc.dram_tensor("wk", (128, MC * 768), BF, kind="ExternalInput").ap()
    wq = nc.dram_tensor("wq", (128, MC * 768), BF, kind="ExternalInput").ap()
    wv = nc.dram_tensor("wv", (128, KC * DPC), BF, kind="ExternalInput").ap()
    wo = nc.dram_tensor("wo", (128, MC * 768), BF, kind="ExternalInput").ap()
    ident = nc.dram_tensor("ident", (128, 128), BF, kind="ExternalInput").ap()
    out1 = nc.dram_tensor("out1", (T, D), FP, kind="ExternalOutput").ap()
    out2 = nc.dram_tensor("out2", (256, D), BF, kind="ExternalOutput").ap()
    from contextlib import ExitStack

    with TileContext(nc) as tc, ExitStack() as st:
        emit_mha(tc, xt, wk, wq, wv, wo, ident, out1, out2, st)
    nc.compile()
    _PROGRAM = nc
    return nc


def _pack_kq(w):
    # [768 d_in, 384 dout] -> [128 p, (m, c, 128)] with d_in = c*128+p
    return np.ascontiguousarray(
        w.reshape(KC, 128, MC, 128).transpose(1, 2, 0, 3).reshape(128, MC * 768)
    ).astype(BF_NP)


def make_in_maps(x, Wq, Wk, Wv, Wo):
    x = np.asarray(x, dtype=np.float32)
    ident = np.eye(128, dtype=np.float32).astype(BF_NP)
    in_maps = []
    xTs = []
    for b in range(B):
        xb = x[b].T  # [768, 1024]
        xTs.append(
            np.ascontiguousarray(
                xb.reshape(KC, 128, T).transpose(1, 0, 2).reshape(128, KC * T)
            ).astype(BF_NP)
        )
    for core in range(NCORES):
        b, hh = core // 2, core % 2
        sl = slice(hh * DPC, (hh + 1) * DPC)
        wvT = np.asarray(Wv)[sl].T.astype(np.float32)  # [768, 384]
        woT = np.asarray(Wo)[:, sl].T.astype(np.float32)  # [384, 768]
        in_maps.append(
            {
                "xt": xTs[b],
                "wq": _pack_kq((np.asarray(Wq)[sl] * 0.125).T.astype(np.float32)),
                "wk": _pack_kq(np.asarray(Wk)[sl].T.astype(np.float32)),
                "wv": np.ascontiguousarray(
                    wvT.reshape(KC, 128, DPC).transpose(1, 0, 2).reshape(128, KC * DPC)
                ).astype(BF_NP),
                "wo": np.ascontiguousarray(
                    woT.reshape(MC, 128, 768).transpose(1, 0, 2).reshape(128, MC * 768)
                ).astype(BF_NP),
                "ident": ident,
            }
        )
    return in_maps


def kernel(x, Wq, Wk, Wv, Wo, bo):
    nc = build_program()
    in_maps = make_in_maps(x, Wq, Wk, Wv, Wo)
    res = run_bass_kernel_spmd(nc, in_maps, core_ids=list(range(NCORES)))
    bo = np.asarray(bo, dtype=np.float32)
    out = np.empty((B, T, D), dtype=np.float32)
    for b in range(B):
        out[b] = res.results[2 * b]["out1"] + res.results[2 * b + 1]["out1"] + bo
        out[b, 768:1024] = (
            np.asarray(res.results[2 * b]["out2"], dtype=np.float32)
            + np.asarray(res.results[2 * b + 1]["out2"], dtype=np.float32)
            + bo
        )
    return out
